# revision 1
# baseline (speedup 1.0000x reference)
"""Distributed multi-head causal attention for 8 TRN2 NeuronCores.

Problem: B=4, T=2048, D=2048, H=16 heads of dk=dv=128.
  out = softmax(mask((q@Wq)(k@Wk)^T / sqrt(dk))) @ (v@Wv) @ Wo

Sharding (2D; all per-core asymmetry lives in host-supplied data so the
SPMD graph is identical on every core):
  core c -> batch b = c//2, head-group g = c%2 (heads 8g..8g+7).
  - QKV projections + attention for (batch b, its 8 heads): fully local.
  - Pair AllGather (replica groups [2b, 2b+1]) exchanges the per-head
    attention outputs (merged^T, bf16) per q-512 chunk; the LAST chunk
    is exchanged per head-pair (4 small collectives) so its output
    projection starts before the final exchange lands.
  - Output projection: each core computes out^T for its batch for HALF
    the output columns (even core: cols 0..1023, odd: 1024..2047). Each
    chunk's projection is emitted after the NEXT chunk's attention so
    the PE stream never parks on an unfinished collective.
  Host reassembles: out[b] = concat(outT_2b, outT_2b+1, axis=0).T

Compute is bf16 on TensorE with f32 PSUM accumulation. Softmax skips the
max-subtraction (scores are ~N(0,1); exp is safe in f32) and obtains the
denominators with an extra ones-matmul so everything stays on TensorE;
causal masking multiplies exp(scores) by a 0/1 triangular tile on the
single diagonal-crossing 128x128 sub-block, and the moving free dim of
diagonal-region matmuls is trimmed to the unmasked columns.

Layouts per core (all bf16 unless noted):
  qT/kT/vT [D=2048, T=2048]   x[b].T            (contraction on partitions)
  wq/wk/wv [D=2048, 1024]     W[:, 1024g:1024(g+1)]
  wo       [2048, 1024]       Wo[:, 1024g:1024(g+1)]
  tri      [128, 128]         causal tile, tri[k, j] = j >= k
  maskT    [2048, 2048]       general mode: mask.T (0/1)
  outT     [1024, 2048] f32   out[b][:, cols].T
Internal: q_s/k_s [8, 128, 2048] (Q^T/K^T per head), v_s [8, 16, 128, 128]
  (V natural, per head per k-block), cc_in/cc_out per (q-chunk, head-pair).
"""
import os
import sys
from contextlib import ExitStack

import numpy as np
import ml_dtypes

import concourse.bass as bass
import concourse.mybir as mybir
import concourse.tile as tile
from concourse import bacc
from concourse.bass_utils import run_bass_kernel_spmd

BF16 = mybir.dt.bfloat16
F32 = mybir.dt.float32

B, T, D = 4, 2048, 2048
H, DK, DV = 16, 128, 128
HG = 8                      # heads per core
N_CORES = 8
QC = 512                    # q-chunk (matmul moving free dim)
NQC = T // QC               # 4
NKB = T // 128              # 16 k-blocks
NDC = D // 128              # 16 contraction chunks
SCALE = 1.0 / np.sqrt(DK)
N_WARM = 170                # dummy matmuls to warm the PE clock gate

_KERNEL_CACHE = {}


class _null_ctx:
    def __enter__(self):
        return None

    def __exit__(self, *a):
        return False


def build_kernel(causal: bool):
    nc = bacc.Bacc("TRN2", num_devices=N_CORES)

    qT = nc.declare_dram_parameter("qT", [D, T], BF16, isOutput=False)
    kT = nc.declare_dram_parameter("kT", [D, T], BF16, isOutput=False)
    vT = nc.declare_dram_parameter("vT", [D, T], BF16, isOutput=False)
    wq = nc.declare_dram_parameter("wq", [D, HG * DK], BF16, isOutput=False)
    wk = nc.declare_dram_parameter("wk", [D, HG * DK], BF16, isOutput=False)
    wv = nc.declare_dram_parameter("wv", [D, HG * DV], BF16, isOutput=False)
    wo = nc.declare_dram_parameter("wo", [H * DV, D // 2], BF16, isOutput=False)
    tri = nc.declare_dram_parameter("tri", [128, 128], BF16, isOutput=False)
    if not causal:
        maskT = nc.declare_dram_parameter("maskT", [T, T], BF16, isOutput=False)
    outT = nc.declare_dram_parameter("outT", [D // 2, T], F32, isOutput=True)

    q_s = nc.dram_tensor("q_s", [HG, 128, T], BF16)
    k_s = nc.dram_tensor("k_s", [HG, 128, T], BF16)
    v_s = nc.dram_tensor("v_s", [HG, 128, NKB * DV], BF16)
    # q-chunks 3..1: one pair-AllGather each (their output projection hides
    # under the next chunk's attention). The LAST chunk (qc 0) uses one small
    # collective per head-pair so its projection can start before the final
    # exchange lands. 2-core replica groups do not support Shared-output
    # collectives; Local output is the supported path (see replica_groups.py).
    QC_ORDER = (2, 1, 0, 3)   # last chunk gets the pipelined sub-gathers
    LAST_QC = QC_ORDER[-1]
    cc_in = {qc: nc.dram_tensor(f"cc_in_{qc}", [128, HG, QC], BF16)
             for qc in QC_ORDER[:-1]}
    cc_out = {qc: nc.dram_tensor(f"cc_out_{qc}", [2, 128, HG, QC], BF16)
              for qc in QC_ORDER[:-1]}
    cc_in0 = [nc.dram_tensor(f"cc_in_l_{j}", [128, 2, QC], BF16) for j in range(4)]
    cc_out0 = [nc.dram_tensor(f"cc_out_l_{j}", [2, 128, 2, QC], BF16) for j in range(4)]
    pair_groups = [[0, 1], [2, 3], [4, 5], [6, 7]]

    def kb_start(qc, kb):
        """First unmasked q column (within the chunk) for this k-block."""
        if not causal:
            return 0
        return min(max((kb - 4 * qc) * 128, 0), QC)

    with tile.TileContext(nc) as tc, ExitStack() as top:
        ent = top.enter_context
        # Pools that live for the whole kernel. The attention input pools are
        # opened BEFORE the projection pools so their SBUF zones never overlap
        # a released projection zone: attention DMAs/matmuls can then overlap
        # the tail of the q-projection instead of serializing behind it.
        consts = ent(tc.tile_pool(name="consts", bufs=1))
        qh_pool = ent(tc.tile_pool(name="qh", bufs=4))
        kh_pool = ent(tc.tile_pool(name="kh", bufs=3 if causal else 2))
        vh_pool = ent(tc.tile_pool(name="vh", bufs=3 if causal else 2))
        pt_pool = ent(tc.tile_pool(name="pt", bufs=8 if causal else 6))
        mstage = ent(tc.tile_pool(name="mstage", bufs=3))
        rinv_pool = ent(tc.tile_pool(name="rinv", bufs=2))
        gm_pool = ent(tc.tile_pool(name="gm", bufs=1)) if not causal else None
        proj_stack = ExitStack()
        w_pool = proj_stack.enter_context(tc.tile_pool(name="w", bufs=3))
        xs_pool = proj_stack.enter_context(tc.tile_pool(name="xs", bufs=2))
        pstage = proj_stack.enter_context(tc.tile_pool(name="pstage", bufs=3))

        ones_sb = consts.tile([128, 128], BF16)
        nc.vector.memset(ones_sb, 1.0)
        tri_sb = consts.tile([128, 128], BF16)
        nc.sync.dma_start(out=tri_sb, in_=tri[:])

        def weight_tile(w_ext, pool):
            """Allocate a weight tile; slices are DMA'd individually (see
            weight_slice) so big weight loads never head-of-line-block the
            latency-sensitive streaming DMAs sharing the queues."""
            return pool.tile([128, NDC, w_ext.shape[1]], BF16, tag="w", name="wtile")

        def weight_slice(w_sb, w_ext, dci):
            nc.sync.dma_start(
                out=w_sb[:, dci, :], in_=w_ext[dci * 128 : (dci + 1) * 128, :]
            )

        wv_sb = weight_tile(wv, w_pool)
        for dci in range(NDC):
            weight_slice(wv_sb, wv, dci)
        wk_sb = weight_tile(wk, w_pool)
        wq_sb = weight_tile(wq, w_pool)

        # ---------------- Phase 1a: V then K projections ----------------
        # V natural ([krows, dv], krows on partitions) per k-block slice of
        # v^T; K^T per head ([dk, q]) per q-chunk slice of k^T. Weights stay
        # resident; x^T streams through small slices.
        with (
            tc.tile_pool(name="warmps", bufs=1, space="PSUM") as warmps,
            tc.tile_pool(name="xv", bufs=4) as xv_pool,
            tc.tile_pool(name="vstage", bufs=2) as vstage,
            tc.tile_pool(name="vpsum", bufs=2, space="PSUM") as vpsum,
            tc.tile_pool(name="kpsum", bufs=2, space="PSUM") as kpsum,
        ):
            # Warm the PE HAM clock gate while the first input DMAs land:
            # dependency-free matmuls on the ones tile into a scratch bank.
            wps = warmps.tile([128, 128], F32)
            for i in range(N_WARM):
                nc.tensor.matmul(
                    wps, lhsT=ones_sb, rhs=ones_sb,
                    start=(i == 0), stop=(i == N_WARM - 1),
                )

            for kb in range(NKB):
                xv = xv_pool.tile([128, NDC, 128], BF16, tag="xv")
                nc.sync.dma_start(
                    out=xv,
                    in_=vT[:, kb * 128 : (kb + 1) * 128].rearrange(
                        "(o p) f -> p o f", p=128
                    ),
                )
                weight_slice(wk_sb, wk, kb)
                ps = vpsum.tile([128, HG * DV], F32, tag="vpsum")
                for dci in range(NDC):
                    for nn in range(2):
                        nc.tensor.matmul(
                            ps[:, nn * 512 : (nn + 1) * 512],
                            lhsT=xv[:, dci, :],
                            rhs=wv_sb[:, dci, nn * 512 : (nn + 1) * 512],
                            start=(dci == 0),
                            stop=(dci == NDC - 1),
                        )
                sb = vstage.tile([128, HG * DV], BF16, tag="vstage")
                nc.vector.tensor_copy(out=sb, in_=ps)
                for h in range(HG):
                    nc.sync.dma_start(
                        out=v_s[h, :, kb * DV : (kb + 1) * DV],
                        in_=sb[:, h * DV : (h + 1) * DV],
                    )

            for qc in range(NQC):
                xs = xs_pool.tile([128, NDC, QC], BF16, tag="xs")
                nc.sync.dma_start(
                    out=xs,
                    in_=kT[:, qc * QC : (qc + 1) * QC].rearrange(
                        "(o p) f -> p o f", p=128
                    ),
                )
                for dci in range(4 * qc, 4 * qc + 4):
                    weight_slice(wq_sb, wq, dci)
                for h in range(HG):
                    ps = kpsum.tile([128, QC], F32, tag="kpsum")
                    for dci in range(NDC):
                        nc.tensor.matmul(
                            ps,
                            lhsT=wk_sb[:, dci, h * 128 : (h + 1) * 128],
                            rhs=xs[:, dci, :],
                            start=(dci == 0),
                            stop=(dci == NDC - 1),
                        )
                    sb = pstage.tile([128, QC], BF16, tag="pstage")
                    nc.vector.tensor_copy(out=sb, in_=ps)
                    nc.sync.dma_start(
                        out=k_s[h, :, qc * QC : (qc + 1) * QC], in_=sb
                    )


        # Attention PSUM pools, opened before the q-projection PSUM pool:
        # s/o/r (6 banks) + q-proj (2 banks) = all 8; the q-projection tail
        # then overlaps attention without any PSUM-zone release dependency.
        # (They reuse the banks of the closed V/K pools; that release dep
        # resolves before attention starts anyway.)
        spsum = ent(tc.tile_pool(name="spsum", bufs=2, space="PSUM"))
        opsum = ent(tc.tile_pool(name="opsum", bufs=2, space="PSUM"))
        rpsum = ent(tc.tile_pool(name="rpsum", bufs=2, space="PSUM"))


        # ---------------- Phase 1b: Q projection ----------------
        # Emitted in the attention q-chunk order so the first attention
        # chunk's inputs land first.
        with tc.tile_pool(name="qpsum", bufs=2, space="PSUM") as qpsum:
            for qc in (3, 2, 1, 0):
                xs = xs_pool.tile([128, NDC, QC], BF16, tag="xs")
                nc.sync.dma_start(
                    out=xs,
                    in_=qT[:, qc * QC : (qc + 1) * QC].rearrange(
                        "(o p) f -> p o f", p=128
                    ),
                )
                for h in range(HG):
                    ps = qpsum.tile([128, QC], F32, tag="qpsum")
                    for dci in range(NDC):
                        nc.tensor.matmul(
                            ps,
                            lhsT=wq_sb[:, dci, h * 128 : (h + 1) * 128],
                            rhs=xs[:, dci, :],
                            start=(dci == 0),
                            stop=(dci == NDC - 1),
                        )
                    sb = pstage.tile([128, QC], BF16, tag="pstage")
                    nc.vector.tensor_copy(out=sb, in_=ps)
                    nc.sync.dma_start(
                        out=q_s[h, :, qc * QC : (qc + 1) * QC], in_=sb
                    )

        proj_stack.close()

        # ---------- Phase 2+3: attention, pair-AG, output proj ----------
        with ExitStack() as phase2:
            ent2 = phase2.enter_context
            wos_pool = ent2(tc.tile_pool(name="wos", bufs=1))
            wo_sb = wos_pool.tile([128, NDC, D // 2], BF16)
            mfq_pool = ent2(tc.tile_pool(name="mfq", bufs=2))
            mf_pool = ent2(tc.tile_pool(name="mf", bufs=4))
            wpart_pool = ent2(tc.tile_pool(name="wpart", bufs=8))
            ob_pool = ent2(tc.tile_pool(name="ob", bufs=3))
            wpsum = ent2(tc.tile_pool(name="wpsum", bufs=2, space="PSUM"))

            def emit_attention(qc):
                nkb = 4 * (qc + 1) if causal else NKB
                if not causal:
                    gm = gm_pool.tile([128, NKB, QC], BF16, tag="gm")
                    nc.sync.dma_start(
                        out=gm,
                        in_=maskT[:, qc * QC : (qc + 1) * QC].rearrange(
                            "(o p) f -> p o f", p=128
                        ),
                    )
                for h in range(HG):
                    if qc == QC_ORDER[0]:
                        weight_slice(wo_sb, wo, 2 * h)
                        weight_slice(wo_sb, wo, 2 * h + 1)
                    qh = qh_pool.tile([128, QC], BF16, tag="qh")
                    nc.sync.dma_start(
                        out=qh, in_=q_s[h, :, qc * QC : (qc + 1) * QC]
                    )
                    kh = kh_pool.tile([128, T], BF16, tag="kh")
                    nc.sync.dma_start(
                        out=kh[:, : nkb * 128], in_=k_s[h, :, : nkb * 128]
                    )
                    vh = vh_pool.tile([128, NKB * DV], BF16, tag="vh")
                    nc.sync.dma_start(
                        out=vh[:, : nkb * DV], in_=v_s[h, :, : nkb * DV]
                    )
                    o_ps = opsum.tile([128, QC], F32, tag="opsum")
                    r_ps = rpsum.tile([128, QC], F32, tag="rpsum")
                    for kb in range(nkb):
                        j0 = kb_start(qc, kb)  # first live q col in chunk
                        s_ps = spsum.tile([128, QC], F32, tag="spsum")
                        nc.tensor.matmul(
                            s_ps[:, j0:],
                            lhsT=kh[:, kb * 128 : (kb + 1) * 128],
                            rhs=qh[:, j0:],
                            start=True,
                            stop=True,
                        )
                        pt = pt_pool.tile([128, QC], BF16, tag="pt")
                        nc.scalar.activation(
                            out=pt[:, j0:],
                            in_=s_ps[:, j0:],
                            func=mybir.ActivationFunctionType.Exp,
                            scale=float(SCALE),
                        )
                        if causal:
                            if j0 < QC and kb - 4 * qc >= 0:
                                # mask the diagonal-crossing 128 columns
                                nc.vector.tensor_mul(
                                    out=pt[:, j0 : j0 + 128],
                                    in0=pt[:, j0 : j0 + 128],
                                    in1=tri_sb,
                                )
                        else:
                            nc.vector.tensor_mul(out=pt, in0=pt, in1=gm[:, kb, :])
                        nc.tensor.matmul(
                            o_ps[:, j0:],
                            lhsT=vh[:, kb * DV : (kb + 1) * DV],
                            rhs=pt[:, j0:],
                            start=(kb == 0),
                            stop=(kb == nkb - 1),
                        )
                        nc.tensor.matmul(
                            r_ps[:, j0:],
                            lhsT=ones_sb,
                            rhs=pt[:, j0:],
                            start=(kb == 0),
                            stop=(kb == nkb - 1),
                        )
                    rinv = rinv_pool.tile([128, QC], F32, tag="rinv")
                    nc.vector.reciprocal(out=rinv, in_=r_ps)
                    msb = mstage.tile([128, QC], BF16, tag="mstage")
                    nc.vector.tensor_mul(out=msb, in0=o_ps, in1=rinv)
                    if qc == LAST_QC:
                        nc.sync.dma_start(
                            out=cc_in0[h // 2][:, h % 2, :], in_=msb
                        )
                        if h % 2 == 1:
                            nc.gpsimd.collective_compute(
                                "AllGather",
                                mybir.AluOpType.bypass,
                                ins=[cc_in0[h // 2][:]],
                                outs=[cc_out0[h // 2][:]],
                                replica_groups=pair_groups,
                            )
                    else:
                        nc.sync.dma_start(out=cc_in[qc][:, h, :], in_=msb)
                        if h == HG - 1:
                            nc.gpsimd.collective_compute(
                                "AllGather",
                                mybir.AluOpType.bypass,
                                ins=[cc_in[qc][:]],
                                outs=[cc_out[qc][:]],
                                replica_groups=pair_groups,
                            )

            def emit_wo(qc):
                # Output projection for this q-chunk.
                if qc != LAST_QC:
                    # Single gather already landed (hides under the next
                    # chunk's attention): straightforward 16-term accumulate.
                    mfq = mfq_pool.tile([128, H, QC], BF16, tag="mfq")
                    nc.sync.dma_start(out=mfq[:, :HG, :], in_=cc_out[qc][0])
                    nc.sync.dma_start(out=mfq[:, HG:, :], in_=cc_out[qc][1])
                    for col in range(D // 2 // 128):
                        w_ps = wpsum.tile([128, QC], F32, tag="wpsum")
                        for hv in range(H):
                            nc.tensor.matmul(
                                w_ps,
                                lhsT=wo_sb[:, hv, col * 128 : (col + 1) * 128],
                                rhs=mfq[:, hv, :],
                                start=(hv == 0),
                                stop=(hv == H - 1),
                            )
                        ob = ob_pool.tile([128, QC], F32, tag="ob")
                        nc.vector.tensor_copy(out=ob, in_=w_ps)
                        nc.sync.dma_start(
                            out=outT[
                                col * 128 : (col + 1) * 128,
                                qc * QC : (qc + 1) * QC,
                            ],
                            in_=ob,
                        )
                else:
                    # Final chunk: accumulate per head-pair quarter so work
                    # starts before the last sub-gather lands.
                    mf = []
                    for j in range(4):
                        mfj = mf_pool.tile([128, 4, QC], BF16, tag="mf")
                        nc.sync.dma_start(out=mfj[:, 0:2, :], in_=cc_out0[j][0])
                        nc.sync.dma_start(out=mfj[:, 2:4, :], in_=cc_out0[j][1])
                        mf.append(mfj)

                    def hv_of(j, t):
                        return 2 * j + t if t < 2 else 8 + 2 * j + (t - 2)

                    parts = []
                    for j in range(4):
                        for col in range(D // 2 // 128):
                            w_ps = wpsum.tile([128, QC], F32, tag="wpsum")
                            for t in range(4):
                                nc.tensor.matmul(
                                    w_ps,
                                    lhsT=wo_sb[
                                        :, hv_of(j, t), col * 128 : (col + 1) * 128
                                    ],
                                    rhs=mf[j][:, t, :],
                                    start=(t == 0),
                                    stop=(t == 3),
                                )
                            if j == 0:
                                part = wpart_pool.tile([128, QC], F32, tag="wpart")
                                nc.vector.tensor_copy(out=part, in_=w_ps)
                                parts.append(part)
                            elif j < 3:
                                nc.vector.tensor_add(
                                    out=parts[col], in0=w_ps, in1=parts[col]
                                )
                            else:
                                ob = ob_pool.tile([128, QC], F32, tag="ob")
                                nc.vector.tensor_add(
                                    out=ob, in0=w_ps, in1=parts[col]
                                )
                                nc.sync.dma_start(
                                    out=outT[
                                        col * 128 : (col + 1) * 128,
                                        LAST_QC * QC : (LAST_QC + 1) * QC,
                                    ],
                                    in_=ob,
                                )

            # Emit each chunk's output projection after the NEXT chunk's
            # attention so the PE instruction stream never parks on a
            # collective that has not landed yet.
            prev = None
            for qc in QC_ORDER:
                emit_attention(qc)
                if prev is not None:
                    emit_wo(prev)
                prev = qc
            emit_wo(prev)

    nc.compile()
    return nc



def kernel(q, k, v, mask, Wq, Wk, Wv, Wo):
    q = np.asarray(q)
    k = np.asarray(k)
    v = np.asarray(v)
    mask = np.asarray(mask)
    causal = bool(np.array_equal(mask, np.tril(np.ones((T, T), dtype=bool))))

    if causal not in _KERNEL_CACHE:
        _KERNEL_CACHE[causal] = build_kernel(causal)
    nc = _KERNEL_CACHE[causal]

    bf = ml_dtypes.bfloat16
    Wq_b = np.asarray(Wq).astype(bf)
    Wk_b = np.asarray(Wk).astype(bf)
    Wv_b = np.asarray(Wv).astype(bf)
    Wo_b = np.asarray(Wo).astype(bf)
    i = np.arange(128)
    tri_np = (i[None, :] >= i[:, None]).astype(bf)  # tri[k, j] = j >= k
    maskT_np = None if causal else np.ascontiguousarray(mask.T).astype(bf)

    in_maps = []
    for c in range(N_CORES):
        b, g = c // 2, c % 2
        m = {
            "qT": np.ascontiguousarray(q[b].T).astype(bf),
            "kT": np.ascontiguousarray(k[b].T).astype(bf),
            "vT": np.ascontiguousarray(v[b].T).astype(bf),
            "wq": np.ascontiguousarray(Wq_b[:, g * 1024 : (g + 1) * 1024]),
            "wk": np.ascontiguousarray(Wk_b[:, g * 1024 : (g + 1) * 1024]),
            "wv": np.ascontiguousarray(Wv_b[:, g * 1024 : (g + 1) * 1024]),
            "wo": np.ascontiguousarray(Wo_b[:, g * 1024 : (g + 1) * 1024]),
            "tri": tri_np,
        }
        if not causal:
            m["maskT"] = maskT_np
        in_maps.append(m)

    trace = bool(os.environ.get("BASS_KERNEL_TRACE")) and (
        "antenv.axon_hooks" in sys.modules
    )
    res = run_bass_kernel_spmd(nc, in_maps, list(range(N_CORES)), trace=trace)
    if trace and res.exec_time_ns is not None:
        print(f"HW exec time: {res.exec_time_ns} ns")
        kernel.last_exec_time_ns = res.exec_time_ns
        kernel.last_results = res

    out = np.empty((B, T, D), dtype=np.float32)
    for b in range(B):
        top = res.results[2 * b]["outT"]        # cols 0..1023, [1024, 2048]
        bot = res.results[2 * b + 1]["outT"]    # cols 1024..2047
        out[b] = np.concatenate([top, bot], axis=0).T
    return out



# revision 8
# speedup vs baseline: 1.0249x; 1.0249x over previous
"""Distributed multi-head causal attention for 8 TRN2 NeuronCores.

Problem: B=4, T=2048, D=2048, H=16 heads of dk=dv=128.
  out = softmax(mask((q@Wq)(k@Wk)^T / sqrt(dk))) @ (v@Wv) @ Wo

Sharding (2D; all per-core asymmetry lives in host-supplied data so the
SPMD graph is identical on every core):
  core c -> batch b = c//2, head-group g = c%2 (heads 8g..8g+7).
  - QKV projections + attention for (batch b, its 8 heads): fully local.
  - Pair AllGather (replica groups [2b, 2b+1]) exchanges the per-head
    attention outputs (merged^T, bf16) per q-chunk.
  - Output projection: each core computes out^T for its batch for HALF
    the output columns (even core: cols 0..1023, odd: 1024..2047).
  Host reassembles: out[b] = concat(outT_2b, outT_2b+1, axis=0).T

Performance structure (v2):
  - All intermediates (Q^T/K^T per head, V natural) stay RESIDENT IN
    SBUF - no DRAM round trip, no attention-phase input DMAs.
  - Two HWDGE rings: weights/cc/output on nc.sync (SP), activation
    streams on nc.scalar (ACT) - no head-of-line blocking.
  - Attention chunks emitted in order (0, 3, 2, 1); each chunk's output
    projection is emitted 2+ chunk-slots after its attention, so every
    pair-AllGather (~25-30us latency) lands long before its consumer.
    Order: att0 att3 att2 wo3 att1 wo2 wo0 wo1.
  - PE never parks: long warmup covers the initial weight DMA, then the
    V/K/Q projections and attention+wo run as one dense matmul stream
    (keeps the HAM clock gate at K=8/8).

Compute is bf16 on TensorE with f32 PSUM accumulation. Softmax skips the
max-subtraction (scores are ~N(0,1); exp is safe in f32) and obtains the
denominators with an extra ones-matmul so everything stays on TensorE;
causal masking multiplies exp(scores) by a 0/1 triangular tile on the
single diagonal-crossing 128x128 sub-block, and the moving free dim of
diagonal-region matmuls is trimmed to the unmasked columns.
"""
import os
import sys
from contextlib import ExitStack

import numpy as np
import ml_dtypes

import concourse.bass as bass
import concourse.mybir as mybir
import concourse.tile as tile
from concourse import bacc
from concourse.bass_utils import run_bass_kernel_spmd

BF16 = mybir.dt.bfloat16
F32 = mybir.dt.float32

B, T, D = 4, 2048, 2048
H, DK, DV = 16, 128, 128
HG = 8                      # heads per core
N_CORES = 8
QC = 512                    # q-chunk (matmul moving free dim)
NQC = T // QC               # 4
NKB = T // 128              # 16 k-blocks
NDC = D // 128              # 16 contraction chunks
SCALE = 1.0 / np.sqrt(DK)
N_WARM = 280                # dummy matmuls to warm the PE clock gate

_KERNEL_CACHE = {}

# Attention chunk emission order and the (attention, wo) interleave.
# Each wo(x) needs its pair-AllGather landed; gathers fire right after
# att(x) and take ~25-35us, so wo(x) is scheduled >=2 slots later.
ATT_ORDER = (0, 3, 2, 1)
EMIT_PLAN = (("att", 0), ("att", 3), ("att", 2), ("wo", 3),
             ("att", 1), ("wo", 2), ("wo", 0), ("wo", 1))


def build_kernel(causal: bool):
    nc = bacc.Bacc("TRN2", num_devices=N_CORES)

    qT = nc.declare_dram_parameter("qT", [D, T], BF16, isOutput=False)
    kT = nc.declare_dram_parameter("kT", [D, T], BF16, isOutput=False)
    vT = nc.declare_dram_parameter("vT", [D, T], BF16, isOutput=False)
    wq = nc.declare_dram_parameter("wq", [D, HG * DK], BF16, isOutput=False)
    wk = nc.declare_dram_parameter("wk", [D, HG * DK], BF16, isOutput=False)
    wv = nc.declare_dram_parameter("wv", [D, HG * DV], BF16, isOutput=False)
    wo = nc.declare_dram_parameter("wo", [H * DV, D // 2], BF16, isOutput=False)
    tri = nc.declare_dram_parameter("tri", [128, 128], BF16, isOutput=False)
    if not causal:
        maskT = nc.declare_dram_parameter("maskT", [T, T], BF16, isOutput=False)
    outT = nc.declare_dram_parameter("outT", [D // 2, T], F32, isOutput=True)

    # Collective staging (collectives require DRAM in/out). One pair
    # AllGather per q-chunk: in [128, HG, QC], out [2, 128, HG, QC]
    # (slot 0 = even core's heads = global heads 0..7).
    cc_in = {qc: nc.dram_tensor(f"cc_in_{qc}", [128, HG, QC], BF16)
             for qc in range(NQC)}
    cc_out = {qc: nc.dram_tensor(f"cc_out_{qc}", [2, 128, HG, QC], BF16)
              for qc in range(NQC)}
    pair_groups = [[0, 1], [2, 3], [4, 5], [6, 7]]

    def kb_start(qc, kb):
        """First unmasked q column (within the chunk) for this k-block."""
        if not causal:
            return 0
        return min(max((kb - 4 * qc) * 128, 0), QC)

    with tile.TileContext(nc) as tc, ExitStack() as top:
        ent = top.enter_context
        consts = ent(tc.tile_pool(name="consts", bufs=1))
        # SBUF-resident per-head projections (live for the whole kernel):
        #   q_all/k_all [128(dk), HG, T] = Q^T/K^T per head
        #   v_all [128(krow), HG, NKB, DV] = V natural per head per k-block
        res_pool = ent(tc.tile_pool(name="res", bufs=1))
        # Phase-1-only pools: the weight ring (two zones, wv->wq reuse)
        # and the double-buffered activation streams. Closed before the
        # attention pools open so their SBUF is recycled.
        proj_stack = ExitStack()
        w_pool = proj_stack.enter_context(tc.tile_pool(name="w", bufs=2))
        x_pool = proj_stack.enter_context(tc.tile_pool(name="xs", bufs=2))

        ones_sb = consts.tile([128, 128], BF16)
        nc.vector.memset(ones_sb, 1.0)
        tri_sb = consts.tile([128, 128], BF16)
        nc.sync.dma_start(out=tri_sb, in_=tri[:])

        q_all = res_pool.tile([128, HG, T], BF16, name="q_all")
        k_all = res_pool.tile([128, HG, T], BF16, name="k_all")
        v_all = res_pool.tile([128, HG, NKB, DV], BF16, name="v_all")

        def weight_tile(pool):
            return pool.tile([128, NDC, HG * 128], BF16, tag="w", name="wtile")

        def weight_slice(w_sb, w_ext, dci):
            nc.sync.dma_start(
                out=w_sb[:, dci, :], in_=w_ext[dci * 128 : (dci + 1) * 128, :]
            )

        # wv first (feeds the first matmuls), wk lands during V proj.
        wv_sb = weight_tile(w_pool)
        for dci in range(NDC):
            weight_slice(wv_sb, wv, dci)
        wk_sb = weight_tile(w_pool)
        wq_sb = None  # allocated after V proj (reuses wv zone)

        def x_stream(src, qc):
            """[128, NDC, QC] slice of an x^T input, contraction on
            partitions, via the ACT HWDGE ring."""
            xs = x_pool.tile([128, NDC, QC], BF16, tag="xs")
            nc.scalar.dma_start(
                out=xs,
                in_=src[:, qc * QC : (qc + 1) * QC].rearrange(
                    "(o p) f -> p o f", p=128
                ),
            )
            return xs

        # ---------------- Phase 1a: V projection ----------------
        # V natural ([krows, dv], krows on partitions): stationary is the
        # x^T block, the weight columns stream.
        with (
            tc.tile_pool(name="warmps", bufs=1, space="PSUM") as warmps,
            tc.tile_pool(name="vpsum", bufs=2, space="PSUM") as vpsum,
        ):
            # Warm the PE HAM clock gate while the first input DMAs land:
            # dependency-free matmuls on the ones tile into a scratch bank.
            wps = warmps.tile([128, 128], F32)
            for i in range(N_WARM):
                nc.tensor.matmul(
                    wps, lhsT=ones_sb, rhs=ones_sb,
                    start=(i == 0), stop=(i == N_WARM - 1),
                )

            for qv in range(NQC):
                xv = x_stream(vT, qv)
                for dci in (range(4) if qv == 0 else []):
                    weight_slice(wk_sb, wk, dci)
                for kbs in range(4):
                    kb = 4 * qv + kbs
                    if kbs == 3 and qv < NQC - 1:
                        for dci in range(4 * (qv + 1), 4 * (qv + 2)):
                            weight_slice(wk_sb, wk, dci)
                    ps = vpsum.tile([128, HG * DV], F32, tag="vpsum")
                    for dci in range(NDC):
                        for nn in range(2):
                            nc.tensor.matmul(
                                ps[:, nn * 512 : (nn + 1) * 512],
                                lhsT=xv[:, dci, kbs * 128 : (kbs + 1) * 128],
                                rhs=wv_sb[:, dci, nn * 512 : (nn + 1) * 512],
                                start=(dci == 0),
                                stop=(dci == NDC - 1),
                            )
                    # drain straight into the resident V tile (strided dst)
                    nc.vector.tensor_copy(out=v_all[:, :, kb, :], in_=ps)

        # ---------------- Phase 1b: K projection ----------------
        # K^T per head ([dk, q]): weight slice stationary, x^T streams.
        wq_sb = weight_tile(w_pool)  # reuses the wv zone
        with tc.tile_pool(name="kpsum", bufs=2, space="PSUM") as kpsum:
            for qc in range(NQC):
                xs = x_stream(kT, qc)
                for dci in range(4 * qc, 4 * qc + 4):
                    weight_slice(wq_sb, wq, dci)
                for h in range(HG):
                    ps = kpsum.tile([128, QC], F32, tag="kpsum")
                    for dci in range(NDC):
                        nc.tensor.matmul(
                            ps,
                            lhsT=wk_sb[:, dci, h * 128 : (h + 1) * 128],
                            rhs=xs[:, dci, :],
                            start=(dci == 0),
                            stop=(dci == NDC - 1),
                        )
                    nc.vector.tensor_copy(
                        out=k_all[:, h, qc * QC : (qc + 1) * QC], in_=ps
                    )

        # ---------------- Phase 1c: Q projection ----------------
        with tc.tile_pool(name="qpsum", bufs=2, space="PSUM") as qpsum:
            for qc in ATT_ORDER:
                xs = x_stream(qT, qc)
                for h in range(HG):
                    ps = qpsum.tile([128, QC], F32, tag="qpsum")
                    for dci in range(NDC):
                        nc.tensor.matmul(
                            ps,
                            lhsT=wq_sb[:, dci, h * 128 : (h + 1) * 128],
                            rhs=xs[:, dci, :],
                            start=(dci == 0),
                            stop=(dci == NDC - 1),
                        )
                    nc.vector.tensor_copy(
                        out=q_all[:, h, qc * QC : (qc + 1) * QC], in_=ps
                    )

        proj_stack.close()

        # ---------- Phase 2+3: attention, pair-AG, output proj ----------
        # wo lands in the SBUF recycled from the weight ring; its 4MB DMA
        # runs under the first attention chunks (first use is emit_wo(3),
        # ~120us into the attention phase).
        wo_pool = ent(tc.tile_pool(name="wop", bufs=1))
        wo_sb = wo_pool.tile([128, NDC, D // 2], BF16, name="wo_sb")
        for dci in range(NDC):
            weight_slice(wo_sb, wo, dci)
        pt_pool = ent(tc.tile_pool(name="pt", bufs=8 if causal else 6))
        mstage = ent(tc.tile_pool(name="mstage", bufs=3))
        rinv_pool = ent(tc.tile_pool(name="rinv", bufs=2))
        mfq_pool = ent(tc.tile_pool(name="mfq", bufs=2 if causal else 1))
        ob_pool = ent(tc.tile_pool(name="ob", bufs=3))
        gm_pool = ent(tc.tile_pool(name="gm", bufs=2)) if not causal else None
        spsum = ent(tc.tile_pool(name="spsum", bufs=2, space="PSUM"))
        opsum = ent(tc.tile_pool(name="opsum", bufs=2, space="PSUM"))
        rpsum = ent(tc.tile_pool(name="rpsum", bufs=2, space="PSUM"))
        wpsum = ent(tc.tile_pool(name="wpsum", bufs=2, space="PSUM"))

        def emit_attention(qc):
            nkb = 4 * (qc + 1) if causal else NKB
            if not causal:
                gm = gm_pool.tile([128, NKB, QC], BF16, tag="gm")
                nc.scalar.dma_start(
                    out=gm,
                    in_=maskT[:, qc * QC : (qc + 1) * QC].rearrange(
                        "(o p) f -> p o f", p=128
                    ),
                )
            for h in range(HG):
                o_ps = opsum.tile([128, QC], F32, tag="opsum")
                r_ps = rpsum.tile([128, QC], F32, tag="rpsum")
                for kb in range(nkb):
                    j0 = kb_start(qc, kb)  # first live q col in chunk
                    s_ps = spsum.tile([128, QC], F32, tag="spsum")
                    nc.tensor.matmul(
                        s_ps[:, j0:],
                        lhsT=k_all[:, h, kb * 128 : (kb + 1) * 128],
                        rhs=q_all[:, h, qc * QC + j0 : (qc + 1) * QC],
                        start=True,
                        stop=True,
                    )
                    pt = pt_pool.tile([128, QC], BF16, tag="pt")
                    nc.scalar.activation(
                        out=pt[:, j0:],
                        in_=s_ps[:, j0:],
                        func=mybir.ActivationFunctionType.Exp,
                        scale=float(SCALE),
                    )
                    if causal:
                        if j0 < QC and kb - 4 * qc >= 0:
                            # mask the diagonal-crossing 128 columns
                            nc.vector.tensor_mul(
                                out=pt[:, j0 : j0 + 128],
                                in0=pt[:, j0 : j0 + 128],
                                in1=tri_sb,
                            )
                    else:
                        nc.vector.tensor_mul(out=pt, in0=pt, in1=gm[:, kb, :])
                    nc.tensor.matmul(
                        o_ps[:, j0:],
                        lhsT=v_all[:, h, kb, :],
                        rhs=pt[:, j0:],
                        start=(kb == 0),
                        stop=(kb == nkb - 1),
                    )
                    nc.tensor.matmul(
                        r_ps[:, j0:],
                        lhsT=ones_sb,
                        rhs=pt[:, j0:],
                        start=(kb == 0),
                        stop=(kb == nkb - 1),
                    )
                rinv = rinv_pool.tile([128, QC], F32, tag="rinv")
                nc.vector.reciprocal(out=rinv, in_=r_ps)
                msb = mstage.tile([128, QC], BF16, tag="mstage")
                nc.vector.tensor_mul(out=msb, in0=o_ps, in1=rinv)
                nc.sync.dma_start(out=cc_in[qc][:, h, :], in_=msb)
                if h == HG - 1:
                    nc.gpsimd.collective_compute(
                        "AllGather",
                        mybir.AluOpType.bypass,
                        ins=[cc_in[qc][:]],
                        outs=[cc_out[qc][:]],
                        replica_groups=pair_groups,
                    )

        def emit_wo(qc):
            # Output projection for this q-chunk; its gather landed long
            # ago (emitted >=2 chunk-slots after emit_attention(qc)).
            mfq = mfq_pool.tile([128, H, QC], BF16, tag="mfq")
            nc.sync.dma_start(out=mfq[:, :HG, :], in_=cc_out[qc][0])
            nc.sync.dma_start(out=mfq[:, HG:, :], in_=cc_out[qc][1])
            for col in range(D // 2 // 128):
                w_ps = wpsum.tile([128, QC], F32, tag="wpsum")
                for hv in range(H):
                    nc.tensor.matmul(
                        w_ps,
                        lhsT=wo_sb[:, hv, col * 128 : (col + 1) * 128],
                        rhs=mfq[:, hv, :],
                        start=(hv == 0),
                        stop=(hv == H - 1),
                    )
                ob = ob_pool.tile([128, QC], F32, tag="ob")
                nc.vector.tensor_copy(out=ob, in_=w_ps)
                nc.sync.dma_start(
                    out=outT[
                        col * 128 : (col + 1) * 128,
                        qc * QC : (qc + 1) * QC,
                    ],
                    in_=ob,
                )

        for kind, qc in EMIT_PLAN:
            if kind == "att":
                emit_attention(qc)
            else:
                emit_wo(qc)

    nc.compile()
    return nc


def kernel(q, k, v, mask, Wq, Wk, Wv, Wo):
    q = np.asarray(q)
    k = np.asarray(k)
    v = np.asarray(v)
    mask = np.asarray(mask)
    causal = bool(np.array_equal(mask, np.tril(np.ones((T, T), dtype=bool))))

    if causal not in _KERNEL_CACHE:
        _KERNEL_CACHE[causal] = build_kernel(causal)
    nc = _KERNEL_CACHE[causal]

    bf = ml_dtypes.bfloat16
    Wq_b = np.asarray(Wq).astype(bf)
    Wk_b = np.asarray(Wk).astype(bf)
    Wv_b = np.asarray(Wv).astype(bf)
    Wo_b = np.asarray(Wo).astype(bf)
    i = np.arange(128)
    tri_np = (i[None, :] >= i[:, None]).astype(bf)  # tri[k, j] = j >= k
    maskT_np = None if causal else np.ascontiguousarray(mask.T).astype(bf)

    in_maps = []
    for c in range(N_CORES):
        b, g = c // 2, c % 2
        m = {
            "qT": np.ascontiguousarray(q[b].T).astype(bf),
            "kT": np.ascontiguousarray(k[b].T).astype(bf),
            "vT": np.ascontiguousarray(v[b].T).astype(bf),
            "wq": np.ascontiguousarray(Wq_b[:, g * 1024 : (g + 1) * 1024]),
            "wk": np.ascontiguousarray(Wk_b[:, g * 1024 : (g + 1) * 1024]),
            "wv": np.ascontiguousarray(Wv_b[:, g * 1024 : (g + 1) * 1024]),
            "wo": np.ascontiguousarray(Wo_b[:, g * 1024 : (g + 1) * 1024]),
            "tri": tri_np,
        }
        if not causal:
            m["maskT"] = maskT_np
        in_maps.append(m)

    trace = bool(os.environ.get("BASS_KERNEL_TRACE")) and (
        "antenv.axon_hooks" in sys.modules
    )
    res = run_bass_kernel_spmd(nc, in_maps, list(range(N_CORES)), trace=trace)
    if trace and res.exec_time_ns is not None:
        print(f"HW exec time: {res.exec_time_ns} ns")
        kernel.last_exec_time_ns = res.exec_time_ns
        kernel.last_results = res

    out = np.empty((B, T, D), dtype=np.float32)
    for b in range(B):
        top = res.results[2 * b]["outT"]        # cols 0..1023, [1024, 2048]
        bot = res.results[2 * b + 1]["outT"]    # cols 1024..2047
        out[b] = np.concatenate([top, bot], axis=0).T
    return out
